# revision 15
# baseline (speedup 1.0000x reference)
"""Trainium2 Bass kernel for nn_AttentionModule (B=8, C=256, HID=32, H=W=64).

Data-parallel over batch: each of the 8 NeuronCores computes one batch
element's full attention:
    q = wq @ xf + bq            [32, 4096]
    k = wk @ xf + bk            [32, 4096]
    v = wv @ xf + bv            [256, 4096]
    scores^T[j, i] = sum_d k[d, j] q[d, i]
    attn = softmax over j (no max subtraction: |scores| <= ~45, exp fits fp32)
    out[c, i] = (sum_j vT[j, c] * exp(scoresT[j, i])) / (sum_j exp(scoresT[j, i]))

Layout choices:
 - scores^T is computed in [j(partition), i(free)] orientation so that the
   A@V matmul needs no transposes (contraction j on partitions for both
   operands).
 - The softmax denominator is computed by a [128x128] all-ones stationary
   matmul over exp(scores^T) chunks, which also broadcasts it across all
   128 partitions for free.
 - q/k are replicated 4x along partitions (qstack/kstack [128, 4096]) so the
   K=32 score matmuls can be issued to 4 distinct PE row-groups
   (tile_position) and run concurrently.
"""

import os
import sys
import math
import functools
from contextlib import ExitStack

import numpy as np

for _p in ("/opt/trn_rl_repo", os.path.expanduser("~/.axon_site/_ro/trn_rl_repo")):
    if os.path.isdir(_p) and _p not in sys.path:
        sys.path.insert(0, _p)

import ml_dtypes  # noqa: E402

import concourse.bass as bass  # noqa: E402
import concourse.tile as tile  # noqa: E402
from concourse import bacc  # noqa: E402
from concourse import mybir  # noqa: E402

B, C, HID, H, W = 8, 256, 32, 64, 64
N = H * W  # 4096
N_CORES = 8

F32 = mybir.dt.float32
F32R = mybir.dt.float32r
BF16 = mybir.dt.bfloat16
F16 = mybir.dt.float16
BF = ml_dtypes.bfloat16

# Precision of the q/k chain (QKV matmuls for q/k + score matmuls).
# "f32r": float32r matmuls (full fp32-ish precision at ~bf16 rate for N>=256)
# "bf16": bf16 matmuls (fast but ~2e-2 rel err from exp amplification)
QK_MODE = os.environ.get("BASS_QK_MODE", "f16")
# Repeat the attention phase R times (timing builds only; output unchanged).
REPEAT = int(os.environ.get("BASS_REPEAT", "1"))
# bisection knobs (timing experiments only — break correctness)
SKIP_AV = os.environ.get("BASS_SKIP_AV") == "1"
SKIP_S = os.environ.get("BASS_SKIP_S") == "1"
SC_BUFS = int(os.environ.get("BASS_SC_BUFS", "1"))
# "po": out[c,i] orientation, separate denominator matmul (3 PE streams)
# "t3": out^T[i,c] orientation with ones-column fused into vT (2 PE streams,
#       plus cheap PE transposes of the output tiles)
ORIENT = os.environ.get("BASS_ORIENT", "t3")
# t3 bisection knobs (timing-only; break correctness)
T3_NO_EXP = os.environ.get("BASS_T3_NO_EXP") == "1"   # pure-PE rate
T3_NO_AV = os.environ.get("BASS_T3_NO_AV") == "1"     # scores+exp rate
# out^T DRAM layout: skip PE transposes, host un-transposes  (t3 only)
OUT_T = os.environ.get("BASS_OUT_T", "1") == "1"
# j-chunks per 16-group computed on DVE via Schraudolph bit-trick exp
# (0 = all 16 on ACT).  Rel-err rises to ~1.3e-2 for any value >= 4.
EXP_DVE = int(os.environ.get("BASS_EXP_DVE", "0"))
# engine for the final normalize (pot * 1/den -> sbuf): "act" or "vec"
NORM_ENG = os.environ.get("BASS_NORM_ENG", "act")
# Schraudolph constants: bf16 = bitcast(int16(round(s*SCHR_A + SCHR_B)))
SCHR_A = 128 * 1.4426950408889634
SCHR_B = (127.0 - 0.0430) * 128
# pipelined head: x DMA in n-slices overlapped with qkv compute (f16 only)
HEAD_PIPE = (os.environ.get("BASS_HEAD_PIPE", "1") == "1"
             and QK_MODE == "f16" and ORIENT == "t3")

IT = int(os.environ.get("BASS_IT", "256"))  # i-tile width
N_ITILES = N // IT
JC = 128          # j-chunk height (partition dim of scores^T chunks)
N_JCHUNK = N // JC            # 32
GRP = 2048 // IT  # j-chunks per exp group (psc tile = 4 PSUM banks)
N_GROUPS = N_JCHUNK // GRP


def build_nc(repeat=None):
    nc = _build_inner(repeat)
    nc.compile()
    return nc


def _build_inner(repeat=None):
    """Build the single-core Bass program (run SPMD on 8 cores)."""
    if repeat is None:
        repeat = REPEAT
    nc = bacc.Bacc("TRN2", target_bir_lowering=False, debug=False)

    qk_dt = {"bf16": BF16, "f16": F16, "f32r": F32R}[QK_MODE]
    qk_dram_dt = F32 if QK_MODE == "f32r" else qk_dt
    x_d = nc.dram_tensor("x", [C, N], F32, kind="ExternalInput").ap()
    wq4_d = nc.dram_tensor("wq4", [128, 2, 128], qk_dram_dt,
                           kind="ExternalInput").ap()
    wk4_d = nc.dram_tensor("wk4", [128, 2, 128], qk_dram_dt,
                           kind="ExternalInput").ap()
    bqk_d = nc.dram_tensor("bqk", [1, 2, 128], qk_dram_dt,
                           kind="ExternalInput").ap()
    wvT_d = nc.dram_tensor("wvT", [128, 2, C], BF16, kind="ExternalInput").ap()
    bv_d = nc.dram_tensor("bv", [1, C], BF16, kind="ExternalInput").ap()
    ident_d = (nc.dram_tensor("ident", [128, 128], BF16,
                              kind="ExternalInput").ap()
               if ORIENT == "t3" and not OUT_T else None)
    if ORIENT == "t3" and OUT_T:
        out_d = nc.dram_tensor("outT", [N, C], F32, kind="ExternalOutput").ap()
    else:
        out_d = nc.dram_tensor("out", [C, N], F32, kind="ExternalOutput").ap()

    with tile.TileContext(nc) as tc, ExitStack() as ctx:
        const = ctx.enter_context(tc.tile_pool(name="const", bufs=1))
        stage = ctx.enter_context(tc.tile_pool(name="stage", bufs=2))
        big = ctx.enter_context(tc.tile_pool(name="big", bufs=1))
        expp = ctx.enter_context(tc.tile_pool(name="expp", bufs=3))
        outp = ctx.enter_context(tc.tile_pool(name="outp", bufs=3))

        # ---- constants / weights -------------------------------------------------
        # f32r operands must be produced by a compute op (rounding); DMA is a
        # bit-copy, so stage fp32 then tensor_copy into the f32r tiles.
        if QK_MODE != "f32r":
            wq4 = const.tile([128, 2, 128], qk_dt)
            nc.sync.dma_start(out=wq4, in_=wq4_d)
            wk4 = const.tile([128, 2, 128], qk_dt)
            nc.sync.dma_start(out=wk4, in_=wk4_d)
        else:
            wq4s = const.tile([128, 2, 128], F32)
            nc.sync.dma_start(out=wq4s, in_=wq4_d)
            wq4 = const.tile([128, 2, 128], F32R)
            nc.vector.tensor_copy(wq4, wq4s)
            wk4s = const.tile([128, 2, 128], F32)
            nc.sync.dma_start(out=wk4s, in_=wk4_d)
            wk4 = const.tile([128, 2, 128], F32R)
            nc.vector.tensor_copy(wk4, wk4s)
        # q/k biases as rows: added into the qk psum via K=1 ones-outer-product
        # matmuls (keeps the psum->sbuf copies at a single sync wait).
        if QK_MODE != "f32r":
            bqk = const.tile([1, 2, 128], qk_dt)
            nc.sync.dma_start(out=bqk, in_=bqk_d)
            ones512 = const.tile([1, 512], qk_dt)
            nc.vector.memset(ones512, 1.0)
        else:
            bqks = const.tile([1, 2, 128], F32)
            nc.sync.dma_start(out=bqks, in_=bqk_d)
            bqk = const.tile([1, 2, 128], F32R)
            nc.vector.tensor_copy(bqk, bqks)
            ones512s = const.tile([1, 512], F32)
            nc.vector.memset(ones512s, 1.0)
            ones512 = const.tile([1, 512], F32R)
            nc.vector.tensor_copy(ones512, ones512s)
        wvT = const.tile([128, 2, C], BF16)
        nc.sync.dma_start(out=wvT, in_=wvT_d)
        bv = const.tile([1, C], BF16)
        nc.sync.dma_start(out=bv, in_=bv_d)
        ones = const.tile([128, 128], BF16)
        nc.vector.memset(ones, 1.0)

        # ---- load x (q/k-chain dtype + bf16 for the v path) ----------------------
        # chunked into 1MB slices on two DMA queues so the QKV matmuls start
        # as soon as the first slice lands instead of after the full 4MB load
        xf = big.tile([128, 2, N], BF16)
        xqk2 = (big.tile([128, 2, N], qk_dt, name="xqk2")
                if QK_MODE != "bf16" else None)
        for ch in range(2):
            xs = stage.tile([128, N], F32)
            eng = nc.sync if ch == 0 else nc.gpsimd
            eng.dma_start(out=xs, in_=x_d[128 * ch:128 * (ch + 1), :])
            if xqk2 is not None:
                nc.vector.tensor_copy(xqk2[:, ch, :], xs)
            nc.vector.tensor_copy(xf[:, ch, :], xs)

        def qk_ap(t):
            return t

        # ---- qstack/kstack [128, N] (q/k replicated 4x on partitions) -----------
        qstack = big.tile([128, N], qk_dt, tag="qstack")
        kstack = big.tile([128, N], qk_dt, tag="kstack")
        VW = C + 1 if ORIENT == "t3" else C
        vT = big.tile([128, N_JCHUNK, VW], BF16, tag="vT")
        if ORIENT == "t3":
            if not OUT_T:
                ident = const.tile([128, 128], BF16)
                nc.sync.dma_start(out=ident, in_=ident_d)
            nc.vector.memset(vT[:, :, C:C + 1], 1.0)
        xqk = xf if QK_MODE == "bf16" else xqk2
        with tc.tile_pool(name="pqkv", bufs=4, space="PSUM") as pqkv:
            for nch in range(N // 512):
                ns = bass.ts(nch, 512)
                pq = pqkv.tile([128, 512], F32, tag="pqkv")
                nc.tensor.matmul(pq, lhsT=qk_ap(wq4[:, 0, :]),
                                 rhs=qk_ap(xqk[:, 0, ns]),
                                 start=True, stop=False)
                nc.tensor.matmul(pq, lhsT=qk_ap(wq4[:, 1, :]),
                                 rhs=qk_ap(xqk[:, 1, ns]),
                                 start=False, stop=False)
                nc.tensor.matmul(pq, lhsT=bqk[:, 0, :], rhs=ones512,
                                 start=False, stop=True)
                nc.vector.tensor_copy(qstack[:, ns], pq)
                pk = pqkv.tile([128, 512], F32, tag="pqkv")
                nc.tensor.matmul(pk, lhsT=qk_ap(wk4[:, 0, :]),
                                 rhs=qk_ap(xqk[:, 0, ns]),
                                 start=True, stop=False)
                nc.tensor.matmul(pk, lhsT=qk_ap(wk4[:, 1, :]),
                                 rhs=qk_ap(xqk[:, 1, ns]),
                                 start=False, stop=False)
                nc.tensor.matmul(pk, lhsT=bqk[:, 1, :], rhs=ones512,
                                 start=False, stop=True)
                nc.vector.tensor_copy(kstack[:, ns], pk)

            # ---- vT [n, c] tiles: vT_sb[128, jc, c] -----------------------------
            for jc in range(N_JCHUNK):
                js = bass.ts(jc, 128)
                pv = pqkv.tile([128, C], F32, tag="pqkv")
                nc.tensor.matmul(pv, lhsT=xf[:, 0, js], rhs=wvT[:, 0, :],
                                 start=True, stop=False)
                nc.tensor.matmul(pv, lhsT=xf[:, 1, js], rhs=wvT[:, 1, :],
                                 start=False, stop=False)
                nc.tensor.matmul(pv, lhsT=ones[0:1, :], rhs=bv,
                                 start=False, stop=True)
                nc.vector.tensor_copy(vT[:, jc, 0:C], pv)

        # ---- attention main loop (t3: fused denominator, out^T) ------------------
        if ORIENT == "t3":
            TIT = 128                     # i-tile width
            TGRP = 16                     # j-chunks per exp group
            TNG = N_JCHUNK // TGRP        # 2 groups per i-tile
            psc = ctx.enter_context(tc.tile_pool(name="psc",
                                                 bufs=2 if T3_NO_AV else 1,
                                                 space="PSUM"))
            if not T3_NO_AV:
                pot = ctx.enter_context(tc.tile_pool(name="pot", bufs=2,
                                                     space="PSUM"))
                if not OUT_T:
                    ptr = ctx.enter_context(tc.tile_pool(name="ptr", bufs=2,
                                                         space="PSUM"))
            pots = {}
            pending = None

            def issue_av_t3(p_rep, p_it, p_g, p_eg):
                p_pot = pots[(p_rep, p_it)]
                for jcl in range(TGRP):
                    jc = p_g * TGRP + jcl
                    nc.tensor.matmul(p_pot, lhsT=p_eg[:, jcl, :],
                                     rhs=vT[:, jc, :],
                                     start=jc == 0, stop=jc == N_JCHUNK - 1)
                if p_g == TNG - 1:
                    isl = bass.ts(p_it, TIT)
                    rcp = outp.tile([128, 1], F32, tag="rcp",
                                    name=f"rcp{p_rep}_{p_it}")
                    nc.vector.reciprocal(rcp, p_pot[:, C:C + 1])
                    if OUT_T:
                        osb = outp.tile([128, C], F32, tag="osb",
                                        name=f"osb{p_rep}_{p_it}")
                        if NORM_ENG == "act":
                            nc.scalar.activation(
                                out=osb, in_=p_pot[:, 0:C],
                                func=mybir.ActivationFunctionType.Copy,
                                scale=rcp)
                        else:
                            nc.vector.tensor_scalar(
                                out=osb, in0=p_pot[:, 0:C],
                                scalar1=rcp, scalar2=None,
                                op0=mybir.AluOpType.mult)
                        nc.sync.dma_start(out=out_d[isl, :], in_=osb)
                        del pots[(p_rep, p_it)]
                        return
                    ots = outp.tile([128, C], BF16, tag="ots",
                                    name=f"ots{p_rep}_{p_it}")
                    nc.vector.tensor_scalar(out=ots, in0=p_pot[:, 0:C],
                                            scalar1=rcp, scalar2=None,
                                            op0=mybir.AluOpType.mult)
                    osb = outp.tile([128, 2, TIT], F32, tag="osb",
                                    name=f"osb{p_rep}_{p_it}")
                    for h in range(2):
                        pt = ptr.tile([128, 128], BF16, tag="pt",
                                      name=f"pt{p_rep}_{p_it}_{h}")
                        nc.tensor.transpose(pt, ots[:, bass.ts(h, 128)], ident)
                        nc.vector.tensor_copy(osb[:, h, :], pt)
                        nc.sync.dma_start(
                            out=out_d[128 * h:128 * (h + 1), isl],
                            in_=osb[:, h, :])
                    del pots[(p_rep, p_it)]

            NGG3 = (N // TIT) * TNG
            for gg in range(NGG3 * repeat + 1):
                if gg < NGG3 * repeat:
                    rep, gg_r = divmod(gg, NGG3)
                    it, g = divmod(gg_r, TNG)
                    isl = bass.ts(it, TIT)
                    if g == 0 and not T3_NO_AV:
                        pots[(rep, it)] = pot.tile([128, C + 1], F32, tag="pot",
                                                   name=f"pot{rep}_{it}")
                    scp = psc.tile([128, TGRP, TIT], F32, tag="scp")
                    # quads hit 4 distinct banks and 4 distinct PE row-groups
                    for q in range(4):
                        for p in range(4):
                            jcl = q + 4 * p
                            jc = g * TGRP + jcl
                            rg = 32 * p
                            nc.tensor.matmul(
                                scp[:, jcl, :],
                                lhsT=qk_ap(kstack[rg:rg + 32, bass.ts(jc, 128)]),
                                rhs=qk_ap(qstack[rg:rg + 32, isl]),
                                start=True, stop=True,
                                tile_position=(rg, 0))
                    if T3_NO_EXP:
                        if not pots.get("egc"):
                            egc = const.tile([128, TGRP, TIT], BF16)
                            nc.vector.memset(egc, 0.001)
                            pots["egc"] = egc
                        eg = pots["egc"]
                    else:
                        eg = expp.tile([128, TGRP, TIT], BF16, tag="eg")
                        m = TGRP - EXP_DVE
                        if m > 0:
                            nc.scalar.activation(
                                out=eg[:, 0:m, :], in_=scp[:, 0:m, :],
                                func=mybir.ActivationFunctionType.Exp)
                        if m < TGRP:
                            egi = eg[:, m:TGRP, :].bitcast(mybir.dt.int16)
                            nc.vector.tensor_scalar(
                                out=egi, in0=scp[:, m:TGRP, :],
                                scalar1=float(SCHR_A), scalar2=float(SCHR_B),
                                op0=mybir.AluOpType.mult,
                                op1=mybir.AluOpType.add)
                if pending is not None and not T3_NO_AV:
                    issue_av_t3(*pending)
                pending = (rep, it, g, eg) if gg < NGG3 * repeat else None
            return nc

        # ---- attention main loop ------------------------------------------------
        psc = ctx.enter_context(tc.tile_pool(name="psc", bufs=SC_BUFS,
                                             space="PSUM"))
        pout = ctx.enter_context(tc.tile_pool(name="pout",
                                              bufs=2 if IT <= 256 else 1,
                                              space="PSUM"))
        psum_s = ctx.enter_context(tc.tile_pool(name="psum_s", bufs=2,
                                                space="PSUM"))
        # Software-pipelined by one exp-group: PE runs group g-1's A.V matmuls
        # (issued after group g's score matmuls) while ACT runs exp(g), so the
        # strict-FIFO PE queue never waits on the in-flight exp.
        tiles = {}
        pending = None  # (rep, it, g, eg)

        def issue_av(p_rep, p_it, p_g, p_eg):
            if SKIP_AV:
                return
            p_po, p_ps = tiles[(p_rep, p_it)]
            for jcl in range(GRP):
                jc = p_g * GRP + jcl
                st = jc == 0
                sp = jc == N_JCHUNK - 1
                # IT<=256: po[:,0] and po[:,1] share one PSUM bank (= one
                # zero region): single start on the bank's first matmul,
                # single stop on its last. IT=512: separate banks, each chain
                # gets its own start/stop.
                sep = IT > 256
                nc.tensor.matmul(p_po[:, 0, :], lhsT=vT[:, jc, 0:128],
                                 rhs=p_eg[:, jcl, :], start=st,
                                 stop=sp if sep else False)
                nc.tensor.matmul(p_po[:, 1, :], lhsT=vT[:, jc, 128:256],
                                 rhs=p_eg[:, jcl, :],
                                 start=st if sep else False, stop=sp)
                if not SKIP_S:
                    nc.tensor.matmul(p_ps, lhsT=ones, rhs=p_eg[:, jcl, :],
                                     start=st, stop=sp)
            if p_g == N_GROUPS - 1:
                isl = bass.ts(p_it, IT)
                rcp = outp.tile([128, IT], F32, tag="rcp")
                if SKIP_S:
                    nc.vector.memset(rcp, 1.0)
                else:
                    nc.vector.reciprocal(rcp, p_ps)
                osb = outp.tile([128, 2, IT], F32, tag="osb")
                nc.vector.tensor_tensor(osb[:, 0, :], p_po[:, 0, :], rcp,
                                        op=mybir.AluOpType.mult)
                nc.vector.tensor_tensor(osb[:, 1, :], p_po[:, 1, :], rcp,
                                        op=mybir.AluOpType.mult)
                nc.sync.dma_start(out=out_d[0:128, isl], in_=osb[:, 0, :])
                nc.sync.dma_start(out=out_d[128:256, isl], in_=osb[:, 1, :])
                del tiles[(p_rep, p_it)]

        NGG = N_ITILES * N_GROUPS
        for gg in range(NGG * repeat + 1):
            if gg < NGG * repeat:
                rep, gg_r = divmod(gg, NGG)
                it, g = divmod(gg_r, N_GROUPS)
                isl = bass.ts(it, IT)
                if g == 0 and not SKIP_AV:
                    tiles[(rep, it)] = (
                        pout.tile([128, 2, IT], F32, tag="po",
                                  name=f"po{rep}_{it}"),
                        psum_s.tile([128, IT], F32, tag="ps",
                                    name=f"ps{rep}_{it}"),
                    )
                scp = psc.tile([128, GRP, IT], F32, tag="scp")
                # concurrent quads hit 4 distinct banks and 4 distinct PE
                # row-groups (IT=256: evens then odds; IT=512: one quad).
                if IT <= 256:
                    quads = [[h + 2 * p for p in range(4)] for h in range(2)]
                else:
                    quads = [list(range(GRP))]
                for quad in quads:
                    for p, jcl in enumerate(quad):
                        jc = g * GRP + jcl
                        rg = 32 * p
                        nc.tensor.matmul(
                            scp[:, jcl, :],
                            lhsT=qk_ap(kstack[rg:rg + 32, bass.ts(jc, 128)]),
                            rhs=qk_ap(qstack[rg:rg + 32, isl]),
                            start=True, stop=True,
                            tile_position=(rg, 0),
                        )
                eg = expp.tile([128, GRP, IT], BF16, tag="eg")
                nc.scalar.activation(out=eg, in_=scp,
                                     func=mybir.ActivationFunctionType.Exp)
            if pending is not None:
                issue_av(*pending)
            pending = (rep, it, g, eg) if gg < NGG * repeat else None

    return nc


def prep_inputs(x, wq, bq, wk, bk, wv, bv):  # noqa: C901
    """Host-side prep: per-core input maps (numpy)."""
    x = np.asarray(x, dtype=np.float32).reshape(B, C, N)
    wq = np.asarray(wq, dtype=np.float32)
    bq = np.asarray(bq, dtype=np.float32)
    wk = np.asarray(wk, dtype=np.float32)
    bk = np.asarray(bk, dtype=np.float32)
    wv = np.asarray(wv, dtype=np.float32)
    bv = np.asarray(bv, dtype=np.float32)

    qk_np = {"bf16": BF, "f16": np.float16, "f32r": np.float32}[QK_MODE]

    def stack4(w):  # [32, 256] -> [128, 2, 128] (4 copies along cols)
        wT = np.ascontiguousarray(w.T)            # [256, 32]
        out = np.empty((128, 2, 128), dtype=qk_np)
        for kc in range(2):
            out[:, kc, :] = np.tile(wT[128 * kc:128 * (kc + 1)], (1, 4))
        return out

    wq4 = stack4(wq)
    wk4 = stack4(wk)
    bqk = np.stack([np.tile(bq, 4), np.tile(bk, 4)])[None].astype(qk_np)
    wvT = np.ascontiguousarray(wv.T)              # [256, 256] = [c_in, c_out]
    wvT_h = np.empty((128, 2, C), dtype=BF)
    for kc in range(2):
        wvT_h[:, kc, :] = wvT[128 * kc:128 * (kc + 1)]
    bv_h = bv[None, :].astype(BF)

    shared = dict(wq4=wq4, wk4=wk4, bqk=bqk, wvT=wvT_h, bv=bv_h)
    if ORIENT == "t3" and not OUT_T:
        shared["ident"] = np.eye(128, dtype=BF)
    return [dict(x=np.ascontiguousarray(x[c]), **shared) for c in range(B)]


@functools.lru_cache(maxsize=4)
def _built_nc(repeat=None):
    return build_nc(repeat)


def run(in_maps, trace=False):
    from concourse.bass_utils import run_bass_kernel_spmd
    nc = _built_nc()
    return run_bass_kernel_spmd(nc, in_maps, core_ids=list(range(N_CORES)),
                                trace=trace)


def kernel(x, wq, bq, wk, bk, wv, bv, _trace=False, _results=None):
    in_maps = prep_inputs(x, wq, bq, wk, bk, wv, bv)
    res = run(in_maps, trace=_trace)
    if _results is not None:
        _results.append(res)
    if ORIENT == "t3" and OUT_T:
        out = np.stack([np.asarray(res.results[c]["outT"],
                                   dtype=np.float32).T for c in range(B)])
    else:
        out = np.stack([np.asarray(res.results[c]["out"], dtype=np.float32)
                        for c in range(B)])
    return out.reshape(B, C, H, W)

